# revision 1
# baseline (speedup 1.0000x reference)
"""AdapterFusionBlock Trainium2 kernel: 8-way batch-parallel, one sample per core.

Self-contained: hardcodes all shapes. Host folds LN affines + adapter scale +
attention scale into weights; window-permutes tokens; per-core Bass/Tile graph
does LN -> qkv+adapter -> window attention (decomposed rel-pos via shift-gather
+ indicator matmuls) -> proj -> shuffle-adapter -> residual -> LN2 -> MLP.
"""
import sys
sys.path.insert(0, '/opt/trn_rl_repo')
import numpy as np
import ml_dtypes
import concourse.bass as bass
import concourse.mybir as mybir
import concourse.tile as tile
from concourse import bacc
from concourse.bass_utils import run_bass_kernel_spmd
from concourse.masks import make_identity

FP32 = mybir.dt.float32
BF16 = mybir.dt.bfloat16
AF = mybir.ActivationFunctionType
ALU = mybir.AluOpType

DIM = 768; NH = 12; HD = 64; WS = 16; B = 8; H = 64; W = 64
MLPD = 4 * DIM; AD = 3 * DIM // 4; HID = DIM // 2
BLOCK_SCALE = 0.5; EPS = 1e-5
T = H * W                  # 4096 tokens per core
NWIN = (H // WS) * (W // WS)   # 16 windows
NT = WS * WS               # 256 tokens per window
CH = 512                   # token chunk for GEMM phases
NCH = T // CH              # 8
SCALE = HD ** -0.5         # 0.125

_BF = ml_dtypes.bfloat16


def _bf16(x):
    return np.ascontiguousarray(x.astype(_BF))


def _col_tiles(v):
    """[n*128] -> [128, n] column layout (col k = channels k*128..k*128+127)."""
    n = v.shape[0] // 128
    return np.ascontiguousarray(v.reshape(n, 128).T.astype(np.float32))


def build_graph():
    import os
    PH = int(os.environ.get("KPHASES", "6"))
    P1CUT = os.environ.get("KP1CUT", "z")
    nc = bacc.Bacc()
    P = 128

    # ---------------- DRAM parameters ----------------
    x_in = nc.declare_dram_parameter("x", [T, DIM], FP32, isOutput=False)
    wqkv = nc.declare_dram_parameter("wqkv", [DIM, 3 * DIM], BF16, isOutput=False)
    a1w = nc.declare_dram_parameter("a1w", [3 * DIM, AD], BF16, isOutput=False)
    a2w = nc.declare_dram_parameter("a2w", [AD, 3 * DIM], BF16, isOutput=False)
    wp = nc.declare_dram_parameter("wp", [DIM, DIM], BF16, isOutput=False)
    wm1 = nc.declare_dram_parameter("wm1", [DIM, MLPD], BF16, isOutput=False)
    wm2 = nc.declare_dram_parameter("wm2", [MLPD, DIM], BF16, isOutput=False)
    relcatT = nc.declare_dram_parameter("relcatT", [HD, 62], BF16, isOutput=False)
    indic = nc.declare_dram_parameter("indic", [32, NT], BF16, isOutput=False)
    # biases / per-channel vectors in column layout [128, n]
    bqkv_c = nc.declare_dram_parameter("bqkv_c", [P, 18], FP32, isOutput=False)
    ba1_c = nc.declare_dram_parameter("ba1_c", [P, 5], FP32, isOutput=False)
    ba2_c = nc.declare_dram_parameter("ba2_c", [P, 18], FP32, isOutput=False)
    bm1_c = nc.declare_dram_parameter("bm1_c", [P, 24], FP32, isOutput=False)
    bp_r = nc.declare_dram_parameter("bp_r", [1, DIM], BF16, isOutput=False)
    bm2_r = nc.declare_dram_parameter("bm2_r", [1, DIM], BF16, isOutput=False)
    w1_c = nc.declare_dram_parameter("w1_c", [P, 6], FP32, isOutput=False)
    b1_c = nc.declare_dram_parameter("b1_c", [P, 6], FP32, isOutput=False)
    w1_r = nc.declare_dram_parameter("w1_r", [1, DIM], FP32, isOutput=False)
    b1_r = nc.declare_dram_parameter("b1_r", [1, DIM], FP32, isOutput=False)
    cw_r = nc.declare_dram_parameter("cw_r", [1, HID], FP32, isOutput=False)
    cb_r = nc.declare_dram_parameter("cb_r", [1, HID], FP32, isOutput=False)
    sw_r = nc.declare_dram_parameter("sw_r", [1, HID], FP32, isOutput=False)
    sb_r = nc.declare_dram_parameter("sb_r", [1, HID], FP32, isOutput=False)
    out_ext = nc.declare_dram_parameter("out", [T, DIM], FP32, isOutput=True)

    # ---------------- DRAM scratch ----------------
    norm_d = nc.dram_tensor("norm_d", [T, DIM], BF16)
    q_d = nc.dram_tensor("q_d", [DIM, T], BF16)
    k_d = nc.dram_tensor("k_d", [DIM, T], BF16)
    v_d = nc.dram_tensor("v_d", [DIM, T], BF16)
    proj_d = nc.dram_tensor("proj_d", [T, DIM], BF16)
    ad_d = nc.dram_tensor("ad_d", [DIM, T], BF16)
    norm2_d = nc.dram_tensor("norm2_d", [T, DIM], BF16)
    t_d = nc.dram_tensor("t_d", [32, P, 744], BF16)
    tshH_d = nc.dram_tensor("tshH_d", [32, P, 713], BF16)
    tshW_d = nc.dram_tensor("tshW_d", [32, P, 729], BF16)

    NTILES = T // P  # 32

    with tile.TileContext(nc) as tc:
        with tc.tile_pool(name="const", bufs=1) as const, \
             tc.tile_pool(name="stats", bufs=1) as stats:
            ident = const.tile([P, P], BF16)
            make_identity(nc, ident[:, :])
            ones1 = const.tile([1, P], BF16)
            nc.vector.memset(ones1[:, :], 1.0)
            ones128 = const.tile([P, 1], BF16)
            nc.vector.memset(ones128[:, :], 1.0)
            eps_col = const.tile([P, 1], FP32)
            nc.vector.memset(eps_col[:, :], EPS)
            relcatT_sb = const.tile([HD, 62], BF16)
            nc.sync.dma_start(out=relcatT_sb[:, :], in_=relcatT[:, :])
            indic_sb = const.tile([32, NT], BF16)
            nc.sync.dma_start(out=indic_sb[:, :], in_=indic[:, :])
            bp_sb = const.tile([1, DIM], BF16)
            nc.sync.dma_start(out=bp_sb[:, :], in_=bp_r[:, :])
            bm2_sb = const.tile([1, DIM], BF16)
            nc.sync.dma_start(out=bm2_sb[:, :], in_=bm2_r[:, :])
            bqkv_sb = const.tile([P, 18], FP32)
            nc.sync.dma_start(out=bqkv_sb[:, :], in_=bqkv_c[:, :])
            ba1_sb = const.tile([P, 5], FP32)
            nc.sync.dma_start(out=ba1_sb[:, :], in_=ba1_c[:, :])
            ba2_sb = const.tile([P, 18], FP32)
            nc.sync.dma_start(out=ba2_sb[:, :], in_=ba2_c[:, :])
            bm1_sb = const.tile([P, 24], FP32)
            nc.sync.dma_start(out=bm1_sb[:, :], in_=bm1_c[:, :])
            w1c_sb = const.tile([P, 6], FP32)
            nc.sync.dma_start(out=w1c_sb[:, :], in_=w1_c[:, :])
            b1c_sb = const.tile([P, 6], FP32)
            nc.sync.dma_start(out=b1c_sb[:, :], in_=b1_c[:, :])
            w1r_sb = const.tile([1, DIM], FP32)
            nc.sync.dma_start(out=w1r_sb[:, :], in_=w1_r[:, :])
            b1r_sb = const.tile([1, DIM], FP32)
            nc.sync.dma_start(out=b1r_sb[:, :], in_=b1_r[:, :])
            cw_sb = const.tile([1, HID], FP32)
            nc.sync.dma_start(out=cw_sb[:, :], in_=cw_r[:, :])
            cb_sb = const.tile([1, HID], FP32)
            nc.sync.dma_start(out=cb_sb[:, :], in_=cb_r[:, :])
            sw_sb = const.tile([1, HID], FP32)
            nc.sync.dma_start(out=sw_sb[:, :], in_=sw_r[:, :])
            sb_sb = const.tile([1, HID], FP32)
            nc.sync.dma_start(out=sb_sb[:, :], in_=sb_r[:, :])

            # LN1 batched stats tiles
            s1m = stats.tile([P, NTILES], FP32)   # mean
            s1r = stats.tile([P, NTILES], FP32)   # rstd
            s2m = stats.tile([P, NTILES], FP32)
            s2r = stats.tile([P, NTILES], FP32)
            # adapter channel stats (accumulated in psum)

            # ============ PHASE 1: LN1 + norm + adapter sums ============
            with tc.tile_pool(name="p1", bufs=4) as p1, \
                 tc.tile_pool(name="p1ps", bufs=1, space="PSUM") as p1ps:
                sum_ps = p1ps.tile([1, DIM], FP32)    # sum_tok(norm)
                sq_ps = p1ps.tile([1, DIM], FP32)     # sum_tok(norm^2)
                for t in range(NTILES):
                    xt = p1.tile([P, DIM], FP32, tag="xt", bufs=4)
                    nc.sync.dma_start(out=xt[:, :], in_=x_in[t * P:(t + 1) * P, :])
                    sm = p1.tile([P, 1], FP32, tag="sm", bufs=4)
                    nc.vector.tensor_reduce(sm[:, :], xt[:, :],
                                            axis=mybir.AxisListType.X, op=ALU.add)
                    scr = p1.tile([P, DIM], BF16, tag="scr", bufs=4)
                    sq = p1.tile([P, 1], FP32, tag="sq", bufs=4)
                    nc.scalar.activation(scr[:, :], xt[:, :], AF.Square,
                                         accum_out=sq[:, :])
                    mean = p1.tile([P, 1], FP32, tag="mean", bufs=4)
                    nc.vector.tensor_scalar(mean[:, :], sm[:, :], 1.0 / DIM, None, op0=ALU.mult)
                    var = p1.tile([P, 1], FP32, tag="var", bufs=4)
                    nc.vector.tensor_scalar(var[:, :], sq[:, :], 1.0 / DIM, None, op0=ALU.mult)
                    m2c = p1.tile([P, 1], FP32, tag="m2c", bufs=4)
                    nc.vector.tensor_tensor(m2c[:, :], mean[:, :], mean[:, :], op=ALU.mult)
                    nc.vector.tensor_tensor(var[:, :], var[:, :], m2c[:, :], op=ALU.subtract)
                    sdv = p1.tile([P, 1], FP32, tag="sdv", bufs=4)
                    nc.scalar.activation(sdv[:, :], var[:, :], AF.Sqrt, bias=eps_col[:, :])
                    rstd = p1.tile([P, 1], FP32, tag="rstd", bufs=4)
                    nc.vector.reciprocal(rstd[:, :], sdv[:, :])
                    nt = p1.tile([P, DIM], BF16, tag="nt", bufs=4)
                    nc.vector.tensor_scalar(nt[:, :], xt[:, :], mean[:, :],
                                            rstd[:, :], op0=ALU.subtract, op1=ALU.mult)
                    nc.scalar.dma_start(out=norm_d[t * P:(t + 1) * P, :], in_=nt[:, :])
                    nsq = p1.tile([P, DIM], BF16, tag="nsq", bufs=4)
                    nc.scalar.activation(nsq[:, :], nt[:, :], AF.Square)
                    for n2 in range(2):
                        sl = slice(n2 * 384, (n2 + 1) * 384)
                        nc.tensor.matmul(sum_ps[:, sl], ones128[:, :], nt[:, sl],
                                         start=(t == 0), stop=(t == NTILES - 1))
                        nc.tensor.matmul(sq_ps[:, sl], ones128[:, :], nsq[:, sl],
                                         start=(t == 0), stop=(t == NTILES - 1))
                # adapter per-channel math (rows [1, *])
                if P1CUT >= "c":
                    Mn = stats.tile([1, DIM], FP32)
                    nc.vector.tensor_scalar(Mn[:, :], sum_ps[:, :], 1.0 / T, None, op0=ALU.mult)
                    Sq = stats.tile([1, DIM], FP32)
                    nc.vector.tensor_scalar(Sq[:, :], sq_ps[:, :], 1.0 / T, None, op0=ALU.mult)
                    # m_chan = w1_0*Mn0 + b1_0 ; sig0 = sigmoid(cw*m+cb); g0 = w1_0*s0, h0 = b1_0*s0
                    mch = stats.tile([1, HID], FP32)
                    nc.vector.tensor_tensor(mch[:, :], w1r_sb[:, 0:HID], Mn[:, 0:HID], op=ALU.mult)
                    nc.vector.tensor_tensor(mch[:, :], mch[:, :], b1r_sb[:, 0:HID], op=ALU.add)
                    sig_in = stats.tile([1, HID], FP32)
                    nc.vector.tensor_tensor(sig_in[:, :], cw_sb[:, :], mch[:, :], op=ALU.mult)
                    nc.vector.tensor_tensor(sig_in[:, :], sig_in[:, :], cb_sb[:, :], op=ALU.add)
                    s0 = stats.tile([1, HID], FP32)
                    nc.scalar.activation(s0[:, :], sig_in[:, :], AF.Sigmoid)
                    g0 = stats.tile([1, HID], FP32)
                    nc.vector.tensor_tensor(g0[:, :], w1r_sb[:, 0:HID], s0[:, :], op=ALU.mult)
                    h0 = stats.tile([1, HID], FP32)
                    nc.vector.tensor_tensor(h0[:, :], b1r_sb[:, 0:HID], s0[:, :], op=ALU.mult)
                    # x1 global stats: u = w1_1*Mn1 + b1_1 ; mu = mean(u)
                    u = stats.tile([1, HID], FP32)
                    nc.vector.tensor_tensor(u[:, :], w1r_sb[:, HID:DIM], Mn[:, HID:DIM], op=ALU.mult)
                    nc.vector.tensor_tensor(u[:, :], u[:, :], b1r_sb[:, HID:DIM], op=ALU.add)
                    mu = stats.tile([1, 1], FP32)
                    nc.vector.tensor_reduce(mu[:, :], u[:, :], axis=mybir.AxisListType.X, op=ALU.add)
                    nc.vector.tensor_scalar(mu[:, :], mu[:, :], 1.0 / HID, None, op0=ALU.mult)
                    # e = w1^2*Sq1 + 2*w1*b1*Mn1 + b1^2 ; E2 = mean(e); var = E2 - mu^2
                    e1 = stats.tile([1, HID], FP32)
                    nc.vector.tensor_tensor(e1[:, :], w1r_sb[:, HID:DIM], w1r_sb[:, HID:DIM], op=ALU.mult)
                    nc.vector.tensor_tensor(e1[:, :], e1[:, :], Sq[:, HID:DIM], op=ALU.mult)
                    e2 = stats.tile([1, HID], FP32)
                    nc.vector.tensor_tensor(e2[:, :], w1r_sb[:, HID:DIM], b1r_sb[:, HID:DIM], op=ALU.mult)
                    nc.vector.tensor_tensor(e2[:, :], e2[:, :], Mn[:, HID:DIM], op=ALU.mult)
                    nc.vector.tensor_scalar(e2[:, :], e2[:, :], 2.0, None, op0=ALU.mult)
                    nc.vector.tensor_tensor(e1[:, :], e1[:, :], e2[:, :], op=ALU.add)
                    e3 = stats.tile([1, HID], FP32)
                    nc.vector.tensor_tensor(e3[:, :], b1r_sb[:, HID:DIM], b1r_sb[:, HID:DIM], op=ALU.mult)
                    nc.vector.tensor_tensor(e1[:, :], e1[:, :], e3[:, :], op=ALU.add)
                    E2 = stats.tile([1, 1], FP32)
                    nc.vector.tensor_reduce(E2[:, :], e1[:, :], axis=mybir.AxisListType.X, op=ALU.add)
                    nc.vector.tensor_scalar(E2[:, :], E2[:, :], 1.0 / HID, None, op0=ALU.mult)
                    mu2 = stats.tile([1, 1], FP32)
                    nc.vector.tensor_tensor(mu2[:, :], mu[:, :], mu[:, :], op=ALU.mult)
                    nc.vector.tensor_tensor(E2[:, :], E2[:, :], mu2[:, :], op=ALU.subtract)
                    rv = stats.tile([1, 1], FP32)
                    nc.scalar.activation(rv[:, :], E2[:, :], AF.Sqrt, bias=eps_col[0:1, :])
                    nc.vector.reciprocal(rv[:, :], rv[:, :])
                    # P = sw*w1_1*rv ; Q = sw*(b1_1 - mu)*rv + sb
                    Pv = stats.tile([1, HID], FP32)
                    nc.vector.tensor_tensor(Pv[:, :], sw_sb[:, :], w1r_sb[:, HID:DIM], op=ALU.mult)
                    nc.vector.tensor_scalar(Pv[:, :], Pv[:, :], rv[:, :], None, op0=ALU.mult)
                    Qv = stats.tile([1, HID], FP32)
                    nc.vector.tensor_scalar(Qv[:, :], b1r_sb[:, HID:DIM], mu[:, :], None, op0=ALU.subtract)
                    nc.vector.tensor_tensor(Qv[:, :], Qv[:, :], sw_sb[:, :], op=ALU.mult)
                    nc.vector.tensor_scalar(Qv[:, :], Qv[:, :], rv[:, :], None, op0=ALU.mult)
                    nc.vector.tensor_tensor(Qv[:, :], Qv[:, :], sb_sb[:, :], op=ALU.add)
                    # reshape row vectors -> column tiles [128, 3] via DRAM bounce
                    vec_d = nc.dram_tensor("vec_d", [4, HID], FP32)
                    nc.sync.dma_start(out=vec_d[0:1, :], in_=g0[0:1, :])
                    nc.sync.dma_start(out=vec_d[1:2, :], in_=h0[0:1, :])
                    nc.sync.dma_start(out=vec_d[2:3, :], in_=Pv[0:1, :])
                    nc.sync.dma_start(out=vec_d[3:4, :], in_=Qv[0:1, :])
                    g0c = stats.tile([P, 3], FP32); h0c = stats.tile([P, 3], FP32)
                    Pc = stats.tile([P, 3], FP32); Qc = stats.tile([P, 3], FP32)
                    for dst, row in ((g0c, 0), (h0c, 1), (Pc, 2), (Qc, 3)):
                        for kk in range(3):
                            src = bass.AP(tensor=vec_d[:, :].tensor,
                                          offset=row * HID + kk * P,
                                          ap=[[1, P], [1, 1]])
                            nc.sync.dma_start(out=dst[:, kk:kk + 1], in_=src)

            # ============ PHASE 2a: qkv + adapter -> q_d/k_d/v_d ============
            if PH >= 2:
             with tc.tile_pool(name="w2a", bufs=1) as w2a, \
                 tc.tile_pool(name="p2a", bufs=2) as p2a, \
                 tc.tile_pool(name="qk2a", bufs=19) as qk2a, \
                 tc.tile_pool(name="ad2a", bufs=6) as ad2a, \
                 tc.tile_pool(name="ps2a", bufs=4, space="PSUM") as ps2a:
                wqkv_sb = [w2a.tile([P, 3 * DIM], BF16, tag="wqkv", bufs=6, name=f"wqkv{_i}") for _i in range(6)]
                for k in range(6):
                    nc.scalar.dma_start(out=wqkv_sb[k][:, :], in_=wqkv[k * P:(k + 1) * P, :])
                a1_sb = [w2a.tile([P, AD], BF16, tag="a1", bufs=18, name=f"a1_{_i}") for _i in range(18)]
                for k in range(18):
                    nc.scalar.dma_start(out=a1_sb[k][:, :], in_=a1w[k * P:(k + 1) * P, :])
                a2_sb = [w2a.tile([P, 3 * DIM], BF16, tag="a2", bufs=5, name=f"a2_{_i}") for _i in range(5)]
                for k in range(5):
                    rows = slice(k * P, min((k + 1) * P, AD))
                    nc.scalar.dma_start(out=a2_sb[k][0:rows.stop - rows.start, :], in_=a2w[rows, :])
                for c in range(NCH):
                    csl = slice(c * CH, (c + 1) * CH)
                    ntc = [p2a.tile([P, CH], BF16, tag="normT", bufs=12, name=f"ntc{_i}") for _i in range(6)]
                    for k in range(6):
                        nc.sync.dma_start(out=ntc[k][:, :], in_=norm_d[csl, k * P:(k + 1) * P],
                                          transpose=True)
                    qkvT = [qk2a.tile([P, CH], BF16, tag="qkvT", bufs=19, name=f"qkvT{_i}") for _i in range(18)]
                    for m in range(18):
                        ps = ps2a.tile([P, CH], FP32, tag="mm")
                        for k in range(6):
                            nc.tensor.matmul(ps[:, :], wqkv_sb[k][:, m * P:(m + 1) * P],
                                             ntc[k][:, :], start=(k == 0), stop=(k == 5))
                        nc.scalar.activation(qkvT[m][:, :], ps[:, :], AF.Identity,
                                             bias=bqkv_sb[:, m:m + 1])
                    ad1T = [ad2a.tile([P, CH], BF16, tag="ad1T", bufs=6, name=f"ad1T{_i}") for _i in range(5)]
                    for m in range(5):
                        rows = min(P, AD - m * P)
                        ps = ps2a.tile([P, CH], FP32, tag="mm")
                        for k in range(18):
                            nc.tensor.matmul(ps[0:rows, :], a1_sb[k][:, m * P:m * P + rows],
                                             qkvT[k][:, :], start=(k == 0), stop=(k == 17))
                        nc.scalar.activation(ad1T[m][0:rows, :], ps[0:rows, :], AF.Gelu,
                                             bias=ba1_sb[0:rows, m:m + 1])
                    for m in range(18):
                        ps = ps2a.tile([P, CH], FP32, tag="mm")
                        for k in range(5):
                            rows = min(P, AD - k * P)
                            nc.tensor.matmul(ps[:, :], a2_sb[k][0:rows, m * P:(m + 1) * P],
                                             ad1T[k][0:rows, :], start=(k == 0), stop=(k == 4))
                        fin = p2a.tile([P, CH], BF16, tag="fin")
                        nc.vector.scalar_tensor_tensor(fin[:, :], ps[:, :], ba2_sb[:, m:m + 1],
                                                       qkvT[m][:, :], op0=ALU.add, op1=ALU.add)
                        dst = (q_d, k_d, v_d)[m // 6]
                        nc.scalar.dma_start(out=dst[(m % 6) * P:(m % 6 + 1) * P, csl],
                                          in_=fin[:, :])

            # ============ PHASE 3: shuffle-adapter elementwise ============
            if PH >= 2:
             with tc.tile_pool(name="p3", bufs=3) as p3:
                for c in range(NCH):
                    csl = slice(c * CH, (c + 1) * CH)
                    for pt in range(3):
                        n0 = p3.tile([P, CH], BF16, tag="n0")
                        nc.scalar.dma_start(out=n0[:, :], in_=norm_d[csl, pt * P:(pt + 1) * P],
                                          transpose=True)
                        a0 = p3.tile([P, CH], BF16, tag="a0")
                        nc.vector.tensor_scalar(a0[:, :], n0[:, :], g0c[:, pt:pt + 1],
                                                h0c[:, pt:pt + 1], op0=ALU.mult, op1=ALU.add)
                        nc.scalar.dma_start(out=ad_d[pt * P:(pt + 1) * P, csl], in_=a0[:, :])
                        n1 = p3.tile([P, CH], BF16, tag="n1")
                        nc.scalar.dma_start(out=n1[:, :], in_=norm_d[csl, HID + pt * P:HID + (pt + 1) * P],
                                          transpose=True)
                        s1t = p3.tile([P, CH], BF16, tag="s1")
                        nc.scalar.activation(s1t[:, :], n1[:, :], AF.Sigmoid,
                                             bias=Qc[:, pt:pt + 1], scale=Pc[:, pt:pt + 1])
                        t1 = p3.tile([P, CH], BF16, tag="t1")
                        nc.vector.tensor_scalar(t1[:, :], n1[:, :], w1c_sb[:, 3 + pt:4 + pt],
                                                b1c_sb[:, 3 + pt:4 + pt], op0=ALU.mult, op1=ALU.add)
                        xs = p3.tile([P, CH], BF16, tag="xs")
                        nc.vector.tensor_tensor(xs[:, :], t1[:, :], s1t[:, :], op=ALU.mult)
                        nc.scalar.dma_start(out=ad_d[HID + pt * P:HID + (pt + 1) * P, csl], in_=xs[:, :])

            # ============ PHASE 2b: windowed attention + proj + residual ============
            xmp_ctx = tc.tile_pool(name="xm", bufs=NTILES + 1)
            xmp = xmp_ctx.__enter__()
            xm_tiles = []
            s2sum = stats.tile([P, NTILES], FP32)
            s2sq = stats.tile([P, NTILES], FP32)
            if PH >= 3:
             with tc.tile_pool(name="w2b", bufs=1) as w2b, \
                 tc.tile_pool(name="p2b", bufs=2) as p2b, \
                 tc.tile_pool(name="psA", bufs=1, space="PSUM") as psA, \
                 tc.tile_pool(name="psB", bufs=1, space="PSUM") as psB, \
                 tc.tile_pool(name="psO", bufs=1, space="PSUM") as psO:
                wp_sb = [w2b.tile([HD, DIM], BF16, tag="wp", bufs=12, name=f"wp{_i}") for _i in range(12)]
                for k in range(12):
                    nc.scalar.dma_start(out=wp_sb[k][:, :], in_=wp[k * HD:(k + 1) * HD, :])
                for w in range(NWIN):
                    wsl = slice(w * NT, (w + 1) * NT)
                    q_sb = p2b.tile([HD, NH * NT], BF16, tag="q")
                    src = bass.AP(tensor=q_d[:, :].tensor, offset=w * NT,
                                  ap=[[T, HD], [HD * T, NH], [1, NT]])
                    nc.sync.dma_start(out=q_sb[:, :], in_=src)
                    k_sb = p2b.tile([HD, NH * NT], BF16, tag="k")
                    src = bass.AP(tensor=k_d[:, :].tensor, offset=w * NT,
                                  ap=[[T, HD], [HD * T, NH], [1, NT]])
                    nc.sync.dma_start(out=k_sb[:, :], in_=src)
                    v_sb = [p2b.tile([P, DIM], BF16, tag="v", bufs=2, name=f"v{_i}") for _i in range(2)]
                    for qt in range(2):
                        nc.sync.dma_start(out=v_sb[qt][:, :],
                                          in_=v_d[:, w * NT + qt * P: w * NT + (qt + 1) * P],
                                          transpose=True)
                    attn_sb = [p2b.tile([HD, NH * P], BF16, tag="attn", bufs=2, name=f"attn{_i}") for _i in range(2)]
                    for qt in range(2):
                        qtw = w * 2 + qt
                        qsl = slice(qt * P, (qt + 1) * P)
                        # --- T matmuls: [128q, (h, 62)] ---
                        t_ps = psA.tile([P, 1536], FP32, tag="A")
                        for h in range(NH):
                            nc.tensor.matmul(t_ps[:, h * 62:(h + 1) * 62],
                                             q_sb[:, h * NT + qt * P: h * NT + (qt + 1) * P],
                                             relcatT_sb[:, :], start=True, stop=True)
                        t_sb = p2b.tile([P, 744], BF16, tag="tsb")
                        nc.scalar.copy(t_sb[:, :], t_ps[:, 0:744])
                        nc.scalar.dma_start(out=t_d[qtw, :, :], in_=t_sb[:, :])
                        # --- shift DMAs (dram->dram) ---
                        goff = 8 * qt
                        srcH = bass.AP(tensor=t_d[:, :, :].tensor,
                                       offset=qtw * P * 744 + goff,
                                       ap=[[16 * 744 + 1, 8], [744, 16], [1, 713]])
                        nc.sync.dma_start(out=tshH_d[qtw, :, :], in_=srcH)
                        srcW = bass.AP(tensor=t_d[:, :, :].tensor,
                                       offset=qtw * P * 744,
                                       ap=[[16 * 744, 8], [744 + 1, 16], [1, 729]])
                        nc.sync.dma_start(out=tshW_d[qtw, :, :], in_=srcW)
                        # --- interleaved reads -> T_s [128, (h, e, 16)] ---
                        t_s = p2b.tile([P, NH * 32], BF16, tag="ts")
                        srcH2 = bass.AP(tensor=tshH_d[:, :, :].tensor,
                                        offset=qtw * P * 713,
                                        ap=[[713, P], [62, NH], [1, 16]])
                        dstH = bass.AP(tensor=t_s.tensor, offset=t_s[:, :].offset,
                                       ap=[[t_s.tensor.shape[1], P], [32, NH], [1, 16]])
                        nc.sync.dma_start(out=dstH, in_=srcH2)
                        srcW2 = bass.AP(tensor=tshW_d[:, :, :].tensor,
                                        offset=qtw * P * 729 + 31,
                                        ap=[[729, P], [62, NH], [1, 16]])
                        dstW = bass.AP(tensor=t_s.tensor, offset=t_s[:, :].offset + 16,
                                       ap=[[t_s.tensor.shape[1], P], [32, NH], [1, 16]])
                        nc.sync.dma_start(out=dstW, in_=srcW2)
                        # --- transposes -> EHWT [32, (h, 128)] ---
                        ehwt_ps = psB.tile([32, NH * P], BF16, tag="B")
                        for h in range(NH):
                            nc.tensor.transpose(ehwt_ps[:, h * P:(h + 1) * P],
                                                t_s[:, h * 32:(h + 1) * 32], ident[:, :])
                        ehwt = p2b.tile([32, NH * P], BF16, tag="ehwt")
                        nc.scalar.copy(ehwt[:, :], ehwt_ps[:, :])
                        # --- S + bias matmuls, exp, sums ---
                        p_sb = p2b.tile([P, NH * NT], BF16, tag="p")
                        for half in range(2):
                            s_ps = psA.tile([P, 1536], FP32, tag="A")
                            for hh in range(6):
                                h = half * 6 + hh
                                nc.tensor.matmul(s_ps[:, hh * NT:(hh + 1) * NT],
                                                 q_sb[:, h * NT + qt * P: h * NT + (qt + 1) * P],
                                                 k_sb[:, h * NT:(h + 1) * NT],
                                                 start=True, stop=False)
                                nc.tensor.matmul(s_ps[:, hh * NT:(hh + 1) * NT],
                                                 ehwt[:, h * P:(h + 1) * P],
                                                 indic_sb[:, :], start=False, stop=True)
                            nc.scalar.activation(p_sb[:, half * 1536:(half + 1) * 1536],
                                                 s_ps[:, :], AF.Exp)
                        sums = p2b.tile([P, NH], FP32, tag="sums")
                        nc.vector.tensor_reduce(
                            sums[:, :],
                            p_sb[:, :].rearrange("p (h k) -> p h k", h=NH),
                            axis=mybir.AxisListType.X, op=ALU.add)
                        rec = p2b.tile([P, NH], FP32, tag="rec")
                        nc.vector.reciprocal(rec[:, :], sums[:, :])
                        for h in range(NH):
                            nc.vector.tensor_scalar(p_sb[:, h * NT:(h + 1) * NT],
                                                    p_sb[:, h * NT:(h + 1) * NT],
                                                    rec[:, h:h + 1], None, op0=ALU.mult)
                        # --- PT transposes + PV ---
                        pt_sb = p2b.tile([P, 2 * NH * P], BF16, tag="pt")
                        for kc in range(2):
                            pt_ps = psB.tile([P, NH * P], BF16, tag="B")
                            for h in range(NH):
                                nc.tensor.transpose(pt_ps[:, h * P:(h + 1) * P],
                                                    p_sb[:, h * NT + kc * P: h * NT + (kc + 1) * P],
                                                    ident[:, :])
                            nc.scalar.copy(pt_sb[:, kc * 1536:(kc + 1) * 1536], pt_ps[:, :])
                        o_ps = psO.tile([HD, NH * P], FP32, tag="O")
                        for h in range(NH):
                            for kc in range(2):
                                nc.tensor.matmul(o_ps[:, h * P:(h + 1) * P],
                                                 v_sb[kc][:, h * HD:(h + 1) * HD],
                                                 pt_sb[:, kc * 1536 + h * P: kc * 1536 + (h + 1) * P],
                                                 start=(kc == 0), stop=(kc == 1))
                        nc.scalar.copy(attn_sb[qt][:, :], o_ps[:, :])
                    # --- proj (token-major) per qtile ---
                    for qt in range(2):
                        pr_ps = psA.tile([P, DIM], FP32, tag="A")
                        for n2, nsl in ((0, slice(0, 512)), (1, slice(512, 768))):
                            for h in range(NH):
                                nc.tensor.matmul(pr_ps[:, nsl], attn_sb[qt][:, h * P:(h + 1) * P],
                                                 wp_sb[h][:, nsl], start=(h == 0), stop=False)
                            nc.tensor.matmul(pr_ps[:, nsl], ones1[:, :], bp_sb[:, nsl],
                                             start=False, stop=True)
                        # residual: xm = x + proj + 0.5*shuffle(adapter), fused here
                        tglob = w * 2 + qt
                        tsl = slice(tglob * P, (tglob + 1) * P)
                        xt = p2b.tile([P, DIM], FP32, tag="xres", bufs=2)
                        nc.sync.dma_start(out=xt[:, :], in_=x_in[tsl, :])
                        adt = p2b.tile([P, DIM], BF16, tag="adt", bufs=2)
                        nc.sync.dma_start(out=adt[:, :], in_=ad_d[:, tsl], transpose=True)
                        pr_sb = p2b.tile([P, DIM], FP32, tag="prsb", bufs=2)
                        nc.scalar.copy(pr_sb[:, :], pr_ps[:, :])
                        t1 = p2b.tile([P, DIM], FP32, tag="t1r", bufs=2)
                        nc.vector.tensor_tensor(t1[:, :], xt[:, :], pr_sb[:, :], op=ALU.add)
                        xm = xmp.tile([P, DIM], BF16, tag="xm", bufs=NTILES + 1,
                                      name=f"xm{w}_{qt}")
                        ad_shuf = bass.AP(tensor=adt.tensor, offset=adt[:, :].offset,
                                          ap=[[adt.tensor.shape[1], P], [1, 384], [384, 2]])
                        xm_v = bass.AP(tensor=xm.tensor, offset=xm[:, :].offset,
                                       ap=[[xm.tensor.shape[1], P], [2, 384], [1, 2]])
                        t1_v = bass.AP(tensor=t1.tensor, offset=t1[:, :].offset,
                                       ap=[[t1.tensor.shape[1], P], [2, 384], [1, 2]])
                        nc.vector.scalar_tensor_tensor(xm_v, ad_shuf, BLOCK_SCALE, t1_v,
                                                       op0=ALU.mult, op1=ALU.add)
                        xm_tiles.append(xm)
                        nc.vector.tensor_reduce(s2sum[:, tglob:tglob + 1], xm[:, :],
                                                axis=mybir.AxisListType.X, op=ALU.add)
                        scr2 = p2b.tile([P, DIM], BF16, tag="scr2", bufs=2)
                        nc.scalar.activation(scr2[:, :], xm[:, :], AF.Square,
                                             accum_out=s2sq[:, tglob:tglob + 1])

            # ============ norm2 tail (batched rstd2 + normalize) ============
            if PH >= 5:
             with tc.tile_pool(name="p4", bufs=3) as p4:
                nc.vector.tensor_scalar(s2m[:, :], s2sum[:, :], 1.0 / DIM, None, op0=ALU.mult)
                m2 = p4.tile([P, NTILES], FP32)
                nc.vector.tensor_tensor(m2[:, :], s2m[:, :], s2m[:, :], op=ALU.mult)
                vv = p4.tile([P, NTILES], FP32)
                nc.vector.tensor_scalar(vv[:, :], s2sq[:, :], 1.0 / DIM, None, op0=ALU.mult)
                nc.vector.tensor_tensor(vv[:, :], vv[:, :], m2[:, :], op=ALU.subtract)
                sd = p4.tile([P, NTILES], FP32)
                nc.scalar.activation(sd[:, :], vv[:, :], AF.Sqrt, bias=eps_col[:, :])
                nc.vector.reciprocal(s2r[:, :], sd[:, :])
                for t in range(NTILES):
                    n2t = p4.tile([P, DIM], BF16, tag="n2t")
                    nc.vector.tensor_scalar(n2t[:, :], xm_tiles[t][:, :], s2m[:, t:t + 1],
                                            s2r[:, t:t + 1], op0=ALU.subtract, op1=ALU.mult)
                    nc.scalar.dma_start(out=norm2_d[t * P:(t + 1) * P, :], in_=n2t[:, :])

             # ============ PHASE 5: MLP (xm tiles still resident) ============
             if PH >= 6:
                with tc.tile_pool(name="w5", bufs=1) as w5, \
                     tc.tile_pool(name="p5", bufs=2) as p5, \
                     tc.tile_pool(name="h5", bufs=25) as h5, \
                     tc.tile_pool(name="ps5", bufs=2, space="PSUM") as ps5:
                    wm1_sb = [w5.tile([P, MLPD], BF16, tag="wm1", bufs=6, name=f"wm1_{_i}") for _i in range(6)]
                    for k in range(6):
                        nc.scalar.dma_start(out=wm1_sb[k][:, :], in_=wm1[k * P:(k + 1) * P, :])
                    wm2_sb = [w5.tile([P, DIM], BF16, tag="wm2", bufs=24, name=f"wm2_{_i}") for _i in range(24)]
                    for k in range(24):
                        nc.scalar.dma_start(out=wm2_sb[k][:, :], in_=wm2[k * P:(k + 1) * P, :])
                    for c in range(NCH):
                        csl = slice(c * CH, (c + 1) * CH)
                        ntc = [p5.tile([P, CH], BF16, tag="n2T", bufs=12, name=f"n2T{_i}") for _i in range(6)]
                        for k in range(6):
                            nc.sync.dma_start(out=ntc[k][:, :],
                                              in_=norm2_d[csl, k * P:(k + 1) * P], transpose=True)
                        hT = [h5.tile([P, CH], BF16, tag="hT", bufs=25, name=f"hT{_i}") for _i in range(24)]
                        for m in range(24):
                            ps = ps5.tile([P, CH], FP32, tag="mm", bufs=2)
                            for k in range(6):
                                nc.tensor.matmul(ps[:, :], wm1_sb[k][:, m * P:(m + 1) * P],
                                                 ntc[k][:, :], start=(k == 0), stop=(k == 5))
                            nc.scalar.activation(hT[m][:, :], ps[:, :], AF.Gelu,
                                                 bias=bm1_sb[:, m:m + 1])
                        for tt in range(CH // P):
                            tglob = c * (CH // P) + tt
                            ps = ps5.tile([P, DIM], FP32, tag="mm2", bufs=2)
                            for n2, nsl in ((0, slice(0, 512)), (1, slice(512, 768))):
                                for k in range(24):
                                    nc.tensor.matmul(ps[:, nsl],
                                                     hT[k][:, tt * P:(tt + 1) * P],
                                                     wm2_sb[k][:, nsl],
                                                     start=(k == 0), stop=False)
                                nc.tensor.matmul(ps[:, nsl], ones1[:, :], bm2_sb[:, nsl],
                                                 start=False, stop=True)
                            ot = p5.tile([P, DIM], FP32, tag="ot")
                            nc.vector.tensor_tensor(ot[:, :], ps[:, :],
                                                    xm_tiles[tglob][:, :], op=ALU.add)
                            nc.scalar.dma_start(out=out_ext[tglob * P:(tglob + 1) * P, :],
                                              in_=ot[:, :])

            xmp_ctx.__exit__(None, None, None)

    if PH < 6:
        # partial build: emit a dummy output so 'out' exists
        with tile.TileContext(nc) as tc2:
            with tc2.tile_pool(name="dummy", bufs=2) as dp:
                for t in range(4):
                    dt_ = dp.tile([P, DIM], FP32, name=f"dummy{t}")
                    nc.sync.dma_start(out=dt_[:, :], in_=x_in[t * P:(t + 1) * P, :])
                    nc.sync.dma_start(out=out_ext[t * P:(t + 1) * P, :], in_=dt_[:, :])
    nc.finalize()
    return nc


_GRAPH = None


def _window_permute(x):
    # [B, H, W, D] -> [B, T, D] in window-major token order (h-major in window)
    xb = x.reshape(B, H // WS, WS, W // WS, WS, DIM).transpose(0, 1, 3, 2, 4, 5)
    return np.ascontiguousarray(xb.reshape(B, T, DIM))


def _window_unpermute(y):
    yb = y.reshape(B, H // WS, W // WS, WS, WS, DIM).transpose(0, 1, 3, 2, 4, 5)
    return np.ascontiguousarray(yb.reshape(B, H, W, DIM))


def kernel(x, w1, b1, Wqkv, bqkv, A1, ba1, A2, ba2, aw, rel_h, rel_w, Wp, bp,
           cw, cb, sw, sb, w2, b2, Wm1, bm1, Wm2, bm2):
    global _GRAPH
    x = np.asarray(x, np.float32)
    f = lambda a: np.asarray(a, np.float32)
    w1, b1, Wqkv, bqkv = f(w1), f(b1), f(Wqkv), f(bqkv)
    A1, ba1, A2, ba2 = f(A1), f(ba1), f(A2), f(ba2)
    aw = float(np.asarray(aw))
    rel_h, rel_w, Wp, bp = f(rel_h), f(rel_w), f(Wp), f(bp)
    cw, cb, sw, sb = f(cw).ravel(), f(cb).ravel(), f(sw).ravel(), f(sb).ravel()
    w2, b2, Wm1, bm1, Wm2, bm2 = f(w2), f(b2), f(Wm1), f(bm1), f(Wm2), f(bm2)

    # ---- host weight folds ----
    Wqkv_f = w1[:, None] * Wqkv
    bqkv_f = b1 @ Wqkv + bqkv
    ksl = slice(DIM, 2 * DIM)
    Wqkv_f[:, ksl] *= SCALE
    bqkv_k = bqkv_f.copy(); bqkv_k[ksl] *= SCALE
    A1_f = A1.copy(); A1_f[ksl, :] /= SCALE
    A2_f = aw * A2
    ba2_f = aw * ba2
    A2_f[:, ksl] *= SCALE
    ba2_k = ba2_f.copy(); ba2_k[ksl] *= SCALE
    Wm1_f = w2[:, None] * Wm1
    bm1_f = b2 @ Wm1 + bm1
    relcat = np.concatenate([rel_h, rel_w], 0)        # [62, 64]
    relcatT_np = _bf16(relcat.T)                      # [64, 62]
    indic_np = np.zeros((32, NT), np.float32)
    for j in range(16):
        for kh in range(16):
            for kw in range(16):
                if kh == 15 - j:
                    indic_np[j, kh * 16 + kw] = 1.0
                if kw == 15 - j:
                    indic_np[16 + j, kh * 16 + kw] = 1.0

    feeds = {
        "wqkv": _bf16(Wqkv_f), "a1w": _bf16(A1_f), "a2w": _bf16(A2_f),
        "wp": _bf16(Wp), "wm1": _bf16(Wm1_f), "wm2": _bf16(Wm2),
        "relcatT": relcatT_np, "indic": _bf16(indic_np),
        "bqkv_c": _col_tiles(bqkv_k), "ba1_c": _col_tiles(np.pad(ba1, (0, 5 * 128 - AD))),
        "ba2_c": _col_tiles(ba2_k), "bm1_c": _col_tiles(bm1_f),
        "bp_r": _bf16(bp.reshape(1, DIM)), "bm2_r": _bf16(bm2.reshape(1, DIM)),
        "w1_c": _col_tiles(w1), "b1_c": _col_tiles(b1),
        "w1_r": w1.reshape(1, DIM).astype(np.float32),
        "b1_r": b1.reshape(1, DIM).astype(np.float32),
        "cw_r": cw.reshape(1, HID).astype(np.float32),
        "cb_r": cb.reshape(1, HID).astype(np.float32),
        "sw_r": sw.reshape(1, HID).astype(np.float32),
        "sb_r": sb.reshape(1, HID).astype(np.float32),
    }

    xp = _window_permute(x)
    in_maps = [dict(feeds, x=np.ascontiguousarray(xp[i])) for i in range(B)]

    if _GRAPH is None:
        _GRAPH = build_graph()
    import os
    trace = os.environ.get("KTRACE", "0") == "1"
    res = run_bass_kernel_spmd(_GRAPH, in_maps, core_ids=list(range(B)), trace=trace)
    if trace and res.exec_time_ns is not None:
        print(f"HW exec time: {res.exec_time_ns} ns")
    y = np.stack([res.results[i]["out"] for i in range(B)], 0)
    return _window_unpermute(y).astype(np.float32)

